# revision 1
# baseline (speedup 1.0000x reference)
"""EnhancedGCNN Trainium2 kernel: 3-layer GCN + BN + relu + mean-pool + MLP + log_softmax.

Sharding: nodes partitioned across 8 NeuronCores at graph boundaries
(dst-sharding); per-edge gathers from an HBM bf16 node-feature table via the
custom GPSIMD dma_gather on 4 parallel SWDGE queues; segment-sum on TensorE
with host-built fp8 0/1 one-hot matrices into feature-major PSUM windows;
symmetric norm folded into table prescale (src) + PE-broadcast postscale
(dst); W/BN/relu fused after aggregation; one AllGather per layer boundary;
per-core pooling + MLP + log_softmax; host assembles per-graph outputs.
"""

import numpy as np
import ml_dtypes
from contextlib import ExitStack

import concourse.bass as bass
from concourse import bacc
from concourse import bass_isa
import concourse.mybir as mybir
from concourse.bass_utils import run_bass_kernel_spmd

# ----- problem constants -------------------------------------------------
N = 100000
G = 2048
D = 128
BN_EPS = 1e-5
NCORES = 8

NLOC = 13312            # padded local nodes per core (13 supers x 1024)
NSUP = 13
NTILES = NLOC // 128    # 104
TROWS = NLOC * NCORES   # 106496 table rows (remapped layout)
NBUCK = 4               # int16 gather sub-tables of 32768 rows
PAIRW = 256             # PSUM window width
NPAIR = 4               # pairs per super
NBLK = NSUP * NPAIR * NBUCK  # 208 gather blocks per layer
NB = 8                  # gather/S/idx buffer depth (multiple of 4)
GMAX = 512              # max graphs per core
FP8_ONE = 0x38          # float8_e4m3 bit pattern of 1.0

_cache = {}
DEBUG_TAPS = False


# ========================================================================
# Host preprocessing: pure index/structure work (sharding + operator build)
# ========================================================================
def _preprocess(edge_index, batch):
    ei = np.asarray(edge_index).astype(np.int64)
    bat = np.asarray(batch).astype(np.int64)
    loops = np.arange(N, dtype=np.int64)
    src = np.concatenate([ei[0], loops])
    dst = np.concatenate([ei[1], loops])
    deg = np.bincount(dst, minlength=N).astype(np.float32)

    # graph-boundary cuts -> per-core node/graph ranges
    gb = np.searchsorted(bat, np.arange(G + 1))
    cuts_n = [0]
    cuts_g = [0]
    for k in range(1, NCORES):
        tgt = k * N // NCORES
        gi = int(np.argmin(np.abs(gb - tgt)))
        cuts_n.append(int(gb[gi]))
        cuts_g.append(gi)
    cuts_n.append(N)
    cuts_g.append(G)
    cuts_n = np.array(cuts_n, dtype=np.int64)
    cuts_g = np.array(cuts_g, dtype=np.int64)
    nodes_k = np.diff(cuts_n)
    ngraph_k = np.diff(cuts_g)
    assert nodes_k.max() <= NLOC and ngraph_k.max() <= GMAX

    own = lambda n: np.searchsorted(cuts_n, n, side="right") - 1
    e_own = own(dst)
    s_own = own(src)
    src_row = NLOC * s_own + (src - cuts_n[s_own])
    d_loc = dst - cuts_n[e_own]

    s_arr = d_loc // 1024
    p_arr = (d_loc % 1024) // PAIRW
    col_arr = d_loc % PAIRW
    b_arr = src_row >> 15
    i16_arr = (src_row & 32767).astype(np.int16)
    blk_arr = (s_arr * NPAIR + p_arr) * NBUCK + b_arr

    counts = np.zeros((NCORES, NBLK), dtype=np.int64)
    for k in range(NCORES):
        counts[k] = np.bincount(blk_arr[e_own == k], minlength=NBLK)
    nch = np.maximum(np.ceil(counts.max(axis=0) / 128).astype(np.int64), 1)
    ch_off = np.zeros(NBLK + 1, dtype=np.int64)
    ch_off[1:] = np.cumsum(nch)
    tot_ch = int(ch_off[-1])
    nch_max = int(nch.max())

    idx_cols = tot_ch * 8
    sv_cols = tot_ch * 256
    IDX, SV, PV, CNT, DEGO = [], [], [], [], []
    for k in range(NCORES):
        m = e_own == k
        B = blk_arr[m]
        order = np.argsort(B, kind="stable")
        Bs = B[order]
        i16s = i16_arr[m][order]
        cols = col_arr[m][order]
        blk_start = np.zeros(NBLK + 1, dtype=np.int64)
        blk_start[1:] = np.cumsum(counts[k])
        pos = np.arange(len(Bs)) - blk_start[Bs]
        idx = np.zeros((128, idx_cols), dtype=np.int16)
        idx[pos % 16, ch_off[Bs] * 8 + pos // 16] = i16s
        for rep in range(1, 8):
            idx[16 * rep:16 * rep + 16] = idx[0:16]
        IDX.append(idx)
        sv = np.zeros((128, sv_cols), dtype=np.uint8)
        gc = ch_off[Bs] + pos // 128
        sv[pos % 128, gc * 256 + cols] = FP8_ONE
        SV.append(sv)

        pv = np.zeros((128, NTILES * GMAX), dtype=np.uint8)
        i_loc = np.arange(nodes_k[k])
        gloc = bat[cuts_n[k] + i_loc] - cuts_g[k]
        pv[i_loc % 128, (i_loc // 128) * GMAX + gloc] = FP8_ONE
        PV.append(pv)
        cnt = np.ones(GMAX, dtype=np.float32)
        bc = np.bincount(gloc, minlength=ngraph_k[k]).astype(np.float32)
        cnt[:ngraph_k[k]] = bc
        CNT.append(np.ascontiguousarray(cnt.reshape(4, 128).T))

        dego = np.ones((128, NTILES), dtype=np.float32)
        i_t = np.arange(NTILES * 128)
        real = i_t < nodes_k[k]
        dego[i_t[real] % 128, i_t[real] // 128] = deg[cuts_n[k] + i_t[real]]
        DEGO.append(dego)

    njt = TROWS // 128
    deg_rm = np.ones((128, njt), dtype=np.float32)
    rows_j = np.zeros(njt, dtype=np.int64)
    xsrc_j = np.zeros(njt, dtype=np.int64)
    for j in range(njt):
        k2, t = j // NTILES, j % NTILES
        r = int(max(0, min(128, nodes_k[k2] - 128 * t)))
        rows_j[j] = r
        xsrc_j[j] = cuts_n[k2] + 128 * t
        if r > 0:
            deg_rm[:r, j] = deg[cuts_n[k2] + 128 * t: cuts_n[k2] + 128 * t + r]

    cast_tiles = [int(j) for j in np.flatnonzero(rows_j > 0)]
    cast_thr = []
    for b in range(NBUCK):
        lim = 32768 * (b + 1)
        cast_thr.append(sum(1 for j in cast_tiles if 128 * j < lim))

    return dict(
        cuts_n=cuts_n, cuts_g=cuts_g, nodes_k=nodes_k, ngraph_k=ngraph_k,
        nch=nch, ch_off=ch_off, tot_ch=tot_ch, nch_max=nch_max,
        IDX=IDX, SV=SV, PV=PV, CNT=CNT, DEGO=DEGO,
        deg_rm=deg_rm, cast_tiles=cast_tiles, rows_j=rows_j, xsrc_j=xsrc_j,
        cast_thr=cast_thr,
    )


# ========================================================================
# Device program (identical for all 8 cores)
# ========================================================================
VEC_NAMES = [f"{nm}{l}" for l in (1, 2, 3) for nm in
             ("b", "bn_g", "bn_b", "bn_m", "bn_v")] + ["fc1_b", "fc2_b"]


def _build_program(pp):
    nch = pp["nch"]
    ch_off = pp["ch_off"]
    nch_max = int(pp["nch_max"])
    cast_tiles = pp["cast_tiles"]
    rows_j = pp["rows_j"]
    xsrc_j = pp["xsrc_j"]
    cast_thr = pp["cast_thr"]
    njt = TROWS // 128
    n_cast = len(cast_tiles)

    f32 = mybir.dt.float32
    bf16 = mybir.dt.bfloat16
    f8 = mybir.dt.float8e4
    i16 = mybir.dt.int16
    AF = mybir.ActivationFunctionType
    AO = mybir.AluOpType

    nc = bacc.Bacc(None, target_bir_lowering=False, num_swdge_queues=4)

    x_in = nc.declare_dram_parameter("x", [N, D], f32, isOutput=False)
    idx_in = nc.declare_dram_parameter("IDX", [128, pp["tot_ch"] * 8], i16, isOutput=False)
    sv_in = nc.declare_dram_parameter("SV", [128, pp["tot_ch"] * 256], f8, isOutput=False)
    pv_in = nc.declare_dram_parameter("PV", [128, NTILES * GMAX], f8, isOutput=False)
    degrm_in = nc.declare_dram_parameter("DEGRM", [128, njt], f32, isOutput=False)
    dego_in = nc.declare_dram_parameter("DEGO", [128, NTILES], f32, isOutput=False)
    cnt_in = nc.declare_dram_parameter("CNT", [128, 4], f32, isOutput=False)
    w_in = [nc.declare_dram_parameter(f"W{l+1}", [D, D], f32, isOutput=False) for l in range(3)]
    fc1_in = nc.declare_dram_parameter("FC1", [D, D], f32, isOutput=False)
    fc2_in = nc.declare_dram_parameter("FC2", [D, 2], f32, isOutput=False)
    vec_in = {nm: nc.declare_dram_parameter(nm, [128, 1], f32, isOutput=False)
              for nm in VEC_NAMES}
    idf_in = nc.declare_dram_parameter("IDF", [128, 128], f32, isOutput=False)
    idb_in = nc.declare_dram_parameter("IDB", [128, 128], bf16, isOutput=False)
    out_ext = nc.declare_dram_parameter("out", [2, GMAX], f32, isOutput=True)
    if DEBUG_TAPS:
        dxt = nc.declare_dram_parameter("dxt", [2048, D], bf16, isOutput=True)
        dhs = nc.declare_dram_parameter("dhs", [NLOC, D], bf16, isOutput=True)
        dht = nc.declare_dram_parameter("dht", [2048, D], bf16, isOutput=True)
        dpool = nc.declare_dram_parameter("dpool", [128, GMAX], f32, isOutput=True)
        dz = nc.declare_dram_parameter("dz", [128, GMAX], f32, isOutput=True)
        df = nc.declare_dram_parameter("df", [2, GMAX], f32, isOutput=True)
        dagg = nc.declare_dram_parameter("dagg", [128, 1024], f32, isOutput=True)
        ddbc = nc.declare_dram_parameter("ddbc", [128, 1024], f32, isOutput=True)

    xt = nc.dram_tensor("xt", [TROWS, D], bf16)
    ht = nc.dram_tensor("ht", [TROWS, D], bf16, addr_space="Shared")
    hs = nc.dram_tensor("hs", [NLOC, D], bf16)

    NCONSTS = 2 + 3 + 1 + 1 + len(VEC_NAMES) + 3  # sync const loads below

    with ExitStack() as ctx:
        sb = lambda nm, shp, dt: ctx.enter_context(nc.sbuf_tensor(nm, shp, dt))
        psum = lambda nm, shp, dt: ctx.enter_context(nc.psum_tensor(nm, shp, dt))
        sem = lambda nm: ctx.enter_context(nc.semaphore(nm))

        gbuf = sb("gbuf", [128, NB, nch_max, D], bf16)
        dgb_sb = sb("dgb_sb", [128, 4, D], bf16)
        dix_sb = sb("dix_sb", [128, 8], i16)
        s_sb = sb("s_sb", [128, NB, nch_max * 256], f8)
        ix_sb = sb("ix_sb", [128, NB, nch_max * 8], i16)
        agg_sb = sb("agg_sb", [128, 1024], f32)
        dbc_sb = sb("dbc_sb", [128, 1024], f32)
        rhs_sb = sb("rhs_sb", [128, 1024], f32)
        hfm_sb = sb("hfm_sb", [128, 1024], bf16)
        hsc_sb = sb("hsc_sb", [128, 1024], bf16)
        htr_sb = sb("htr_sb", [128, 2, 8, 128], bf16)
        xtile_sb = sb("xtile_sb", [128, 2, 128], f32)
        xc_sb = sb("xc_sb", [128, 2, 128], bf16)
        p_sb = sb("p_sb", [128, 2, GMAX], f8)
        pool_sb = sb("pool_sb", [128, GMAX], f32)
        z_sb = sb("z_sb", [128, GMAX], f32)
        f_sb = sb("f_sb", [2, GMAX], f32)
        pm_sb = sb("pm_sb", [2, GMAX], f32)
        d_sb = sb("d_sb", [2, GMAX], f32)
        e_sb = sb("e_sb", [2, GMAX], f32)
        ls_sb = sb("ls_sb", [2, GMAX], f32)
        o_sb = sb("o_sb", [2, GMAX], f32)
        ones_sb = sb("ones_sb", [128, 128], f32)
        idf_sb = sb("idf_sb", [128, 128], f32)
        idb_sb = sb("idb_sb", [128, 128], bf16)
        w_sb = [sb(f"w{l}_sb", [128, 128], f32) for l in range(3)]
        fc1_sb = sb("fc1_sb", [128, 128], f32)
        fc2_sb = sb("fc2_sb", [128, 2], f32)
        vec_sb = {nm: sb(nm + "_sb", [128, 1], f32) for nm in VEC_NAMES}
        degrm_sb = sb("degrm_sb", [128, njt], f32)
        dego_sb = sb("dego_sb", [128, NTILES], f32)
        cnt_sb = sb("cnt_sb", [128, 4], f32)
        dinvrm_sb = sb("dinvrm_sb", [128, njt], f32)
        dinvo_sb = sb("dinvo_sb", [128, NTILES], f32)
        rcnt_sb = sb("rcnt_sb", [128, 4], f32)
        scale_sb = [sb(f"scale{l}_sb", [128, 1], f32) for l in range(3)]
        bias_sb = [sb(f"bias{l}_sb", [128, 1], f32) for l in range(3)]

        psA = psum("psA", [128, 1024], f32)
        psW = psum("psW", [128, 1024], f32)
        psT = psum("psT", [128, 2, 1024], bf16)
        psP = psum("psP", [128, GMAX], f32)

        ld_c = sem("ld_c")
        ac_su = sem("ac_su")
        dv_pre = sem("dv_pre")
        dv_su = sem("dv_su")
        c_ld = sem("c_ld")
        c_st = sem("c_st")
        ld_i = sem("ld_i")
        ld_s = sem("ld_s")
        ld_p = sem("ld_p")
        g_q = [sem(f"g_q{q}") for q in range(4)]
        pe_blk = sem("pe_blk")
        dv_rhs = sem("dv_rhs")
        pe_bc = sem("pe_bc")
        dv_bc = sem("dv_bc")
        dv_agg = sem("dv_agg")
        pe_w = sem("pe_w")
        ac_ep = sem("ac_ep")
        dv_h = sem("dv_h")
        pe_tr = sem("pe_tr")
        dv_tr = sem("dv_tr")
        st_tab = sem("st_tab")
        pe_pool = sem("pe_pool")
        cc = sem("cc")
        t_pe = sem("t_pe")
        t_dv = sem("t_dv")
        t_ac = sem("t_ac")
        t_gp = sem("t_gp")
        io = sem("io")

        with nc.Block() as block:

            # ------------- SYNC: streaming loads + table writes -------------
            @block.sync
            def _(sync):
                nload = 0
                def ld(dst_ap, src_ap):
                    nonlocal nload
                    sync.dma_start(dst_ap, src_ap).then_inc(ld_c, 16)
                    nload += 1
                ld(idf_sb[:], idf_in[:])
                ld(idb_sb[:], idb_in[:])
                for l in range(3):
                    ld(w_sb[l][:], w_in[l][:])
                ld(fc1_sb[:], fc1_in[:])
                ld(fc2_sb[:], fc2_in[:])
                for nm in VEC_NAMES:
                    ld(vec_sb[nm][:], vec_in[nm][:])
                ld(degrm_sb[:], degrm_in[:])
                ld(dego_sb[:], dego_in[:])
                ld(cnt_sb[:], cnt_in[:])
                assert nload == NCONSTS, nload

                for l in range(3):
                    if DEBUG_TAPS and l == 0:
                        sync.wait_ge(c_st, n_cast * 16)
                        sync.dma_start(dxt[:], xt[0:2048, :]).then_inc(io, 16)
                    if DEBUG_TAPS and l == 0:
                        pass
                    if DEBUG_TAPS and l == 1:
                        sync.wait_ge(st_tab, NSUP * 16)
                        sync.dma_start(dhs[:], hs[:]).then_inc(io, 16)
                        sync.wait_ge(cc, 1)
                        sync.dma_start(dht[:], ht[0:2048, :]).then_inc(io, 16)
                    for s in range(NSUP):
                        for kb in range(NPAIR * NBUCK):
                            k = s * NPAIR * NBUCK + kb
                            kk = l * NBLK + k
                            nchk = int(nch[k])
                            if kk >= NB:
                                sync.wait_ge(pe_blk, kk - NB + 1)
                            off = int(ch_off[k])
                            sync.dma_start(
                                ix_sb[:, kk % NB, 0:nchk * 8],
                                idx_in[:, off * 8: off * 8 + nchk * 8],
                            ).then_inc(ld_i, 16)
                            sync.dma_start(
                                s_sb[:, kk % NB, 0:nchk * 256],
                                sv_in[:, off * 256: off * 256 + nchk * 256],
                            ).then_inc(ld_s, 16)
                        if DEBUG_TAPS and l == 0 and s == 1:
                            sync.wait_ge(dv_bc, 1)
                            sync.dma_start(ddbc[:], dbc_sb[:]).then_inc(io, 16)
                            sync.wait_ge(dv_agg, 1)
                            sync.dma_start(dagg[:], agg_sb[:]).then_inc(io, 16)
                        if l < 2:
                            # write table shard for super s-1 (and s=12 after loop)
                            if s >= 1:
                                sp = s - 1
                                sync.wait_ge(dv_tr, (l * NSUP + sp) * 8 + 8)
                                sync.dma_start(
                                    hs[1024 * sp: 1024 * (sp + 1), :].rearrange(
                                        "(t p) f -> p t f", p=128),
                                    htr_sb[:, sp % 2],
                                ).then_inc(st_tab, 16)
                        else:
                            for t8 in range(8):
                                t = s * 8 + t8
                                if t >= 2:
                                    sync.wait_ge(pe_pool, t - 1)
                                sync.dma_start(
                                    p_sb[:, t % 2, :],
                                    pv_in[:, t * GMAX: (t + 1) * GMAX],
                                ).then_inc(ld_p, 16)
                    if l < 2:
                        sp = NSUP - 1
                        sync.wait_ge(dv_tr, (l * NSUP + sp) * 8 + 8)
                        sync.dma_start(
                            hs[1024 * sp: 1024 * (sp + 1), :].rearrange(
                                "(t p) f -> p t f", p=128),
                            htr_sb[:, sp % 2],
                        ).then_inc(st_tab, 16)

                if DEBUG_TAPS:
                    sync.wait_ge(t_dv, 2)
                    sync.dma_start(dpool[:], pool_sb[:]).then_inc(io, 16)
                    sync.wait_ge(t_ac, 1)
                    sync.dma_start(dz[:], z_sb[:]).then_inc(io, 16)
                    sync.wait_ge(t_ac, 2)
                    sync.dma_start(df[:], f_sb[:]).then_inc(io, 16)
                sync.wait_ge(t_dv, 4)
                sync.dma_start(out_ext[:], o_sb[:]).then_inc(io, 16)

            # ------------- SCALAR: setup sqrt, x-cast, epilogues, tail ------
            @block.scalar
            def _(scalar):
                scalar.wait_ge(ld_c, NCONSTS * 16)
                scalar.wait_ge(dv_pre, 1)
                scalar.activation(dinvrm_sb[:], degrm_sb[:], AF.Sqrt)
                scalar.activation(dinvo_sb[:], dego_sb[:], AF.Sqrt)
                for l in range(3):
                    inst = scalar.activation(scale_sb[l][:], scale_sb[l][:], AF.Sqrt)
                inst.then_inc(ac_su, 1)

                # x-cast pipeline (prefetched loads)
                scalar.wait_ge(dv_su, 1)
                j0 = cast_tiles[0]
                scalar.dma_start(
                    xtile_sb[0:int(rows_j[j0]), 0, :],
                    x_in[int(xsrc_j[j0]): int(xsrc_j[j0]) + int(rows_j[j0]), :],
                ).then_inc(c_ld, 16)
                for i, j in enumerate(cast_tiles):
                    r = int(rows_j[j])
                    if i + 1 < n_cast:
                        jn = cast_tiles[i + 1]
                        rn = int(rows_j[jn])
                        scalar.dma_start(
                            xtile_sb[0:rn, (i + 1) % 2, :],
                            x_in[int(xsrc_j[jn]): int(xsrc_j[jn]) + rn, :],
                        ).then_inc(c_ld, 16)
                    scalar.wait_ge(c_ld, (i + 1) * 16)
                    if i >= 2:
                        scalar.wait_ge(c_st, (i - 1) * 16)
                    scalar.activation(
                        xc_sb[0:r, i % 2, :], xtile_sb[0:r, i % 2, :],
                        AF.Copy, scale=dinvrm_sb[0:r, j: j + 1],
                    )
                    scalar.dma_start(
                        xt[128 * j: 128 * j + r, :], xc_sb[0:r, i % 2, :]
                    ).then_inc(c_st, 16)

                # per-super BN+relu epilogues
                for l in range(3):
                    for s in range(NSUP):
                        g = l * NSUP + s
                        scalar.wait_ge(pe_w, g + 1)
                        if g >= 1:
                            if (g - 1) // NSUP < 2:
                                scalar.wait_ge(dv_h, g)
                            else:
                                scalar.wait_ge(pe_tr, 8 * g)
                        scalar.activation(
                            hfm_sb[:], psW[:], AF.Relu,
                            bias=bias_sb[l][:], scale=scale_sb[l][:],
                        ).then_inc(ac_ep, 1)

                # tail
                scalar.wait_ge(t_pe, 2)
                scalar.activation(z_sb[:], psA[:, 0:GMAX], AF.Relu,
                                  bias=vec_sb["fc1_b"][:]).then_inc(t_ac, 1)
                scalar.wait_ge(t_pe, 3)
                scalar.activation(f_sb[:], psW[0:2, 512:512 + GMAX], AF.Identity,
                                  bias=vec_sb["fc2_b"][0:2, :]).then_inc(t_ac, 1)
                scalar.wait_ge(t_dv, 3)
                scalar.activation(e_sb[:], d_sb[:], AF.Exp).then_inc(t_ac, 1)
                scalar.wait_ge(t_gp, 2)
                scalar.activation(ls_sb[:], pm_sb[:], AF.Ln).then_inc(t_ac, 1)

            # ------------- VECTOR: setup, evictions, scaling ----------------
            @block.vector
            def _(vector):
                vector.wait_ge(ld_c, NCONSTS * 16)
                for l in range(3):
                    inst = vector.tensor_scalar_add(scale_sb[l][:],
                                                    vec_sb[f"bn_v{l+1}"][:], BN_EPS)
                inst.then_inc(dv_pre, 1)
                vector.wait_ge(ac_su, 1)
                vector.reciprocal(dinvrm_sb[:], dinvrm_sb[:])
                vector.reciprocal(dinvo_sb[:], dinvo_sb[:])
                vector.reciprocal(rcnt_sb[:], cnt_sb[:])
                vector.memset(ones_sb[:], 1.0)
                for l in range(3):
                    vector.reciprocal(scale_sb[l][:], scale_sb[l][:])
                    vector.tensor_tensor(out=scale_sb[l][:], in0=scale_sb[l][:],
                                         in1=vec_sb[f"bn_g{l+1}"][:], op=AO.mult)
                    vector.tensor_tensor(out=bias_sb[l][:], in0=vec_sb[f"b{l+1}"][:],
                                         in1=vec_sb[f"bn_m{l+1}"][:], op=AO.subtract)
                    vector.tensor_tensor(out=bias_sb[l][:], in0=bias_sb[l][:],
                                         in1=scale_sb[l][:], op=AO.mult)
                    inst = vector.tensor_tensor(out=bias_sb[l][:], in0=bias_sb[l][:],
                                                in1=vec_sb[f"bn_b{l+1}"][:], op=AO.add)
                inst.then_inc(dv_su, 1)

                base = idf_sb[:]
                idf_mid8 = bass.AP(base.tensor, base.offset,
                                   [base.ap[0], [0, 8], base.ap[1]])
                idf_mid4 = bass.AP(base.tensor, base.offset,
                                   [base.ap[0], [0, 4], base.ap[1]])

                for l in range(3):
                    for s in range(NSUP):
                        g = l * NSUP + s
                        if DEBUG_TAPS and l == 0 and s == 1:
                            vector.wait_ge(io, 48)
                        if g >= 1:
                            vector.wait_ge(pe_bc, g)
                        vector.tensor_tensor(
                            out=rhs_sb[:].rearrange("p (t f) -> p t f", t=8),
                            in0=dinvo_sb[:, 8 * s: 8 * s + 8].to_broadcast([128, 8, 128]),
                            in1=idf_mid8, op=AO.mult,
                        ).then_inc(dv_rhs, 1)
                        vector.wait_ge(pe_bc, g + 1)
                        vector.tensor_copy(dbc_sb[:], psW[:]).then_inc(dv_bc, 1)
                        vector.wait_ge(pe_blk, l * NBLK + (s + 1) * NPAIR * NBUCK)
                        if g >= 1:
                            vector.wait_ge(pe_w, g)
                        vector.tensor_tensor(out=agg_sb[:], in0=psA[:],
                                             in1=dbc_sb[:], op=AO.mult
                                             ).then_inc(dv_agg, 1)
                        if l < 2:
                            vector.wait_ge(ac_ep, g + 1)
                            if g >= 1:
                                vector.wait_ge(pe_tr, 8 * g)
                            vector.tensor_tensor(out=hsc_sb[:], in0=hfm_sb[:],
                                                 in1=dbc_sb[:], op=AO.mult
                                                 ).then_inc(dv_h, 1)
                        for t in range(8):
                            vector.wait_ge(pe_tr, g * 8 + t + 1)
                            if t == 0 and g >= 2:
                                gp = g - 2
                                if gp // NSUP < 2:
                                    vector.wait_ge(st_tab, (gp + 1) * 16)
                                else:
                                    vector.wait_ge(pe_pool, ((gp % NSUP) + 1) * 8)
                            vector.tensor_copy(
                                htr_sb[:, s % 2, t, :], psT[:, t % 2, 0:128]
                            ).then_inc(dv_tr, 1)

                # tail
                vector.wait_ge(pe_bc, 3 * NSUP)
                vector.tensor_tensor(
                    out=rhs_sb[:, 0:GMAX].rearrange("p (t f) -> p t f", t=4),
                    in0=rcnt_sb[:].to_broadcast([128, 4, 128]),
                    in1=idf_mid4, op=AO.mult,
                ).then_inc(dv_rhs, 1)
                vector.wait_ge(t_pe, 1)
                vector.tensor_copy(dbc_sb[:, 0:GMAX], psW[:, 0:GMAX]).then_inc(t_dv, 1)
                vector.wait_ge(pe_pool, NTILES)
                vector.tensor_tensor(out=pool_sb[:], in0=psP[:],
                                     in1=dbc_sb[:, 0:GMAX], op=AO.mult
                                     ).then_inc(t_dv, 1)
                vector.wait_ge(t_gp, 1)
                vector.tensor_tensor(out=d_sb[:], in0=f_sb[:], in1=pm_sb[:],
                                     op=AO.subtract).then_inc(t_dv, 1)
                vector.wait_ge(t_ac, 4)
                vector.tensor_tensor(out=o_sb[:], in0=d_sb[:], in1=ls_sb[:],
                                     op=AO.subtract).then_inc(t_dv, 1)

            # ------------- GPSIMD: gathers + collectives + partition ops ----
            @block.gpsimd
            def _(gpsimd):
                for l in range(3):
                    table = xt if l == 0 else ht
                    if l >= 1:
                        gpsimd.wait_ge(cc, l)
                    for k in range(NBLK):
                        kk = l * NBLK + k
                        nchk = int(nch[k])
                        ni = nchk * 128
                        b = k % NBUCK
                        if l == 0:
                            gpsimd.wait_ge(c_st, cast_thr[b] * 16)
                        if kk >= NB:
                            gpsimd.wait_ge(pe_blk, kk - NB + 1)
                        gpsimd.wait_ge(ld_i, (kk + 1) * 16)
                        rows_b = min(32768, TROWS - 32768 * b)
                        gpsimd.dma_gather(
                            gbuf[:, kk % NB, 0:nchk, :],
                            table[32768 * b: 32768 * b + rows_b, :],
                            ix_sb[:, kk % NB, 0:nchk * 8],
                            ni, ni, D, elem_step=D,
                            queue_num=kk % 4,
                            single_packet=False,
                        ).then_inc(g_q[kk % 4], 16)
                    if l < 2:
                        gpsimd.wait_ge(st_tab, (l + 1) * NSUP * 16)
                        gpsimd.collective_compute(
                            "AllGather", AO.bypass,
                            replica_groups=[list(range(NCORES))],
                            ins=[hs[:]], outs=[ht[:]],
                        ).then_inc(cc, 1)

                gpsimd.wait_ge(t_ac, 2)
                gpsimd.partition_all_reduce(
                    pm_sb[:], f_sb[:], channels=2,
                    reduce_op=bass_isa.ReduceOp.max).then_inc(t_gp, 1)
                gpsimd.wait_ge(t_ac, 3)
                gpsimd.partition_all_reduce(
                    pm_sb[:], e_sb[:], channels=2,
                    reduce_op=bass_isa.ReduceOp.add).then_inc(t_gp, 1)

            # ------------- TENSOR: all matmuls -------------
            @block.tensor
            def _(tensor):
                for l in range(3):
                    for s in range(NSUP):
                        g = l * NSUP + s
                        if g >= 1:
                            tensor.wait_ge(dv_agg, g)
                        for pair in range(NPAIR):
                            for b in range(NBUCK):
                                k = (s * NPAIR + pair) * NBUCK + b
                                kk = l * NBLK + k
                                nchk = int(nch[k])
                                qpl = NBLK // 4
                                tensor.wait_ge(g_q[kk % 4],
                                               (l * qpl + (k // 4) + 1) * 16)
                                tensor.wait_ge(ld_s, (kk + 1) * 16)
                                inst = None
                                for c in range(nchk):
                                    inst = tensor.matmul(
                                        psA[:, PAIRW * pair: PAIRW * (pair + 1)],
                                        lhsT=gbuf[:, kk % NB, c, :],
                                        rhs=s_sb[:, kk % NB, c * 256: c * 256 + 256],
                                        start=(b == 0 and c == 0),
                                        stop=(b == NBUCK - 1 and c == nchk - 1),
                                    )
                                inst.then_inc(pe_blk, 1)
                        tensor.wait_ge(dv_rhs, g + 1)
                        if g >= 1:
                            tensor.wait_ge(ac_ep, g)
                        tensor.matmul(psW[:, 0:512], lhsT=ones_sb[:],
                                      rhs=rhs_sb[:, 0:512], start=True, stop=True)
                        tensor.matmul(psW[:, 512:1024], lhsT=ones_sb[:],
                                      rhs=rhs_sb[:, 512:1024], start=True,
                                      stop=True).then_inc(pe_bc, 1)
                        tensor.wait_ge(dv_agg, g + 1)
                        tensor.wait_ge(dv_bc, g + 1)
                        tensor.matmul(psW[:, 0:512], lhsT=w_sb[l][:],
                                      rhs=agg_sb[:, 0:512], start=True, stop=True)
                        tensor.matmul(psW[:, 512:1024], lhsT=w_sb[l][:],
                                      rhs=agg_sb[:, 512:1024], start=True,
                                      stop=True).then_inc(pe_w, 1)
                        src_tile = hsc_sb if l < 2 else hfm_sb
                        for t in range(8):
                            if l < 2:
                                tensor.wait_ge(dv_h, g + 1)
                            else:
                                tensor.wait_ge(ac_ep, g + 1)
                            if g * 8 + t >= 2:
                                tensor.wait_ge(dv_tr, g * 8 + t - 1)
                            tensor.transpose(
                                psT[:, t % 2, 0:128],
                                in_=src_tile[:, 128 * t: 128 * (t + 1)],
                                identity=idb_sb[:],
                            ).then_inc(pe_tr, 1)
                        if l == 2:
                            for t8 in range(8):
                                t = s * 8 + t8
                                tensor.wait_ge(dv_tr, g * 8 + t8 + 1)
                                tensor.wait_ge(ld_p, (t + 1) * 16)
                                tensor.matmul(
                                    psP[:],
                                    lhsT=htr_sb[:, s % 2, t8, :],
                                    rhs=p_sb[:, t % 2, :],
                                    start=(t == 0), stop=(t == NTILES - 1),
                                ).then_inc(pe_pool, 1)

                # tail matmuls
                tensor.wait_ge(dv_rhs, 3 * NSUP + 1)
                tensor.wait_ge(ac_ep, 3 * NSUP)
                tensor.matmul(psW[:, 0:GMAX], lhsT=ones_sb[:],
                              rhs=rhs_sb[:, 0:GMAX], start=True,
                              stop=True).then_inc(t_pe, 1)
                tensor.wait_ge(t_dv, 2)
                tensor.wait_ge(dv_agg, 3 * NSUP)
                tensor.matmul(psA[:, 0:GMAX], lhsT=fc1_sb[:], rhs=pool_sb[:],
                              start=True, stop=True).then_inc(t_pe, 1)
                tensor.wait_ge(t_ac, 1)
                tensor.matmul(psW[0:2, 512:512 + GMAX], lhsT=fc2_sb[:],
                              rhs=z_sb[:], start=True, stop=True).then_inc(t_pe, 1)

    nc.compile()
    return nc


# ========================================================================
# Entry point
# ========================================================================
def kernel(**inputs):
    edge_index = np.asarray(inputs["edge_index"])
    batch = np.asarray(inputs["batch"])

    key = "prog"
    if key not in _cache:
        pp = _preprocess(edge_index, batch)
        nc = _build_program(pp)
        _cache[key] = (pp, nc)
    pp, nc = _cache[key]

    x = np.ascontiguousarray(np.asarray(inputs["x"], dtype=np.float32))
    f8v = ml_dtypes.float8_e4m3
    common = {
        "x": x,
        "DEGRM": pp["deg_rm"],
        "W1": np.asarray(inputs["W1"], dtype=np.float32),
        "W2": np.asarray(inputs["W2"], dtype=np.float32),
        "W3": np.asarray(inputs["W3"], dtype=np.float32),
        "FC1": np.asarray(inputs["fc1_W"], dtype=np.float32),
        "FC2": np.asarray(inputs["fc2_W"], dtype=np.float32),
        "IDF": np.eye(128, dtype=np.float32),
        "IDB": np.eye(128, dtype=np.float32).astype(ml_dtypes.bfloat16),
    }

    def vec128(v):
        a = np.zeros((128, 1), dtype=np.float32)
        v = np.asarray(v, dtype=np.float32).ravel()
        a[:len(v), 0] = v
        return a

    for l in (1, 2, 3):
        common[f"b{l}"] = vec128(inputs[f"b{l}"])
        common[f"bn_g{l}"] = vec128(inputs[f"bn{l}_g"])
        common[f"bn_b{l}"] = vec128(inputs[f"bn{l}_b"])
        common[f"bn_m{l}"] = vec128(inputs[f"bn{l}_m"])
        common[f"bn_v{l}"] = vec128(inputs[f"bn{l}_v"])
    common["fc1_b"] = vec128(inputs["fc1_b"])
    common["fc2_b"] = vec128(inputs["fc2_b"])

    in_maps = []
    for k in range(NCORES):
        m = dict(common)
        m["IDX"] = pp["IDX"][k]
        m["SV"] = pp["SV"][k].view(f8v)
        m["PV"] = pp["PV"][k].view(f8v)
        m["DEGO"] = pp["DEGO"][k]
        m["CNT"] = pp["CNT"][k]
        in_maps.append(m)

    res = run_bass_kernel_spmd(nc, in_maps, core_ids=list(range(NCORES)),
                               trace=not DEBUG_TAPS)
    kernel.last_exec_time_ns = res.exec_time_ns
    kernel.last_results = res.results

    out = np.zeros((G, 2), dtype=np.float32)
    cuts_g = pp["cuts_g"]
    for k in range(NCORES):
        ngk = int(pp["ngraph_k"][k])
        o = res.results[k]["out"]  # [2, 512]
        out[cuts_g[k]: cuts_g[k] + ngk] = o[:, :ngk].T
    return out


kernel.last_exec_time_ns = None

